# revision 9
# baseline (speedup 1.0000x reference)
"""Causal self-attention (B=4, T=2048, C=1024, H=16, HD=64) on 8 trn2 cores.

Sharding: core = (batch b, head-group g), 8 heads each. Host sums the two
partial output projections per batch and adds b_proj.

v4 kernel (per core):
  A: QK^T projection in fp8e4 + DoubleRow (contraction 256/instr, 2x):
     psum = 32*(x Wqk) (weights pre-scaled x32 on host to dodge e4m3
     subnormals; K bias dropped -- softmax-invariant; Q bias x32 via ACT).
     QT/KT stored bf16 at 32x scale -> exp scale absorbs /1024.
     V projection bf16 -> vaug bf16 [k,(h,d+ones)].
  B: per (q-window, head-pair): S^T[k,q] bf16 matmul pair on PE row-groups
     h0/h64 (runs concurrent); one wide exp (scale 0.125/1024) -> pt bf16;
     diagonal blocks width-reduced + fixed lower-triangle mask; AV bf16
     accumulates per head in PSUM, ones column = softmax denominator row.
     Normalize: DVE copies (tensor_copy shifts partitions; other DVE ops
     can't), recip, gpsimd partition-broadcast, DVE mult -> YT bf16.
  C: out = Y @ Wp (bf16), PSUM -> fp16 SBUF -> HBM; host sums in fp32.
Schedule: input DMAs round-robin over the 4 DGE queues (sync/scalar/
vector/gpsimd) -- one queue is only ~45GB/s; n-major QK groups start the
PE ~3us in; qc0 attention + V proj interleave with late QK groups;
out-projection/V matmuls injected at kc granularity inside attention
loops (2 held back past the last AV to cover the PSUM copy-out); the
last q-chunk runs as two half-windows so its output projection
overlaps attention instead of trailing.
"""

import numpy as np

B, T, C, H, HD = 4, 2048, 1024, 16, 64
G = 2              # head groups (tensor parallel)
HG = H // G        # 8 heads per group
GC = HG * HD       # 512 group channels
P = 128
NQC = T // 512     # 4 q-chunks of 512
NKC = T // P       # 16 k-chunks of 128
KO_C = C // P      # 8 contraction chunks for C=1024
KO2 = C // 256     # 4 DoubleRow contraction chunks
KO_G = GC // P     # 4 contraction chunks for GC=512

_cache = {}


def _build():
    import concourse.bass as bass
    import concourse.tile as tile
    from concourse import bacc, mybir

    f32 = mybir.dt.float32
    f16 = mybir.dt.float16
    bf16 = mybir.dt.bfloat16
    f8 = mybir.dt.float8e4
    DR = mybir.MatmulPerfMode.DoubleRow

    nc = bacc.Bacc(name="csa")
    x8 = nc.declare_dram_parameter("x8", [P, NQC, KO2, 2, 512], f8, isOutput=False)
    xb = nc.declare_dram_parameter("xb", [P, KO_C, T], bf16, isOutput=False)
    wqk8 = nc.declare_dram_parameter("wqk8", [P, 8, KO2, 2, P], f8, isOutput=False)
    bq32 = nc.declare_dram_parameter("bq32", [P, 4], f32, isOutput=False)
    wv = nc.declare_dram_parameter("wv", [P, KO_C, GC], bf16, isOutput=False)
    bv = nc.declare_dram_parameter("bv", [P, GC], f32, isOutput=False)
    wp = nc.declare_dram_parameter("wp", [P, KO_G, C], bf16, isOutput=False)
    mask = nc.declare_dram_parameter("mask", [P, P], bf16, isOutput=False)
    out = nc.declare_dram_parameter("out", [T, C], f16, isOutput=True)

    from contextlib import ExitStack

    with tile.TileContext(nc) as tc, ExitStack() as ctx:
            singles = ctx.enter_context(tc.tile_pool(name="singles", bufs=1))
            ppool = ctx.enter_context(tc.tile_pool(name="ppool", bufs=3))
            spool = ctx.enter_context(tc.tile_pool(name="spool", bufs=4))
            opool = ctx.enter_context(tc.tile_pool(name="opool", bufs=3))
            pp = ctx.enter_context(tc.tile_pool(name="pp", bufs=2, space="PSUM"))
            ps = ctx.enter_context(tc.tile_pool(name="ps", bufs=2, space="PSUM"))
            py = ctx.enter_context(tc.tile_pool(name="py", bufs=2, space="PSUM"))
            # ---- resident tensors ----
            x8big = singles.tile([P, NQC, KO2, 2, 512], f8, tag="x8big")
            xbbig = singles.tile([P, KO_C, T], bf16, tag="xbbig")
            wqk_s = singles.tile([P, 8, KO2, 2, P], f8, tag="wqk_s")
            QT = singles.tile([P, HG // 2, T], bf16, tag="QT")
            KT = singles.tile([P, HG // 2, T], bf16, tag="KT")
            vaug = singles.tile([P, NKC, HG, 65], bf16, tag="vaug")
            YT = singles.tile([P, KO_G, T], bf16, tag="YT")
            tri = singles.tile([P, P], bf16, tag="tri")
            bq_s = singles.tile([P, 4], f32, tag="bq")
            bv_s = singles.tile([P, HG, HD], f32, tag="bv")
            wv_s = singles.tile([P, KO_C, GC], bf16, tag="wv")
            wp_s = singles.tile([P, KO_G, C], bf16, tag="wp")

            ones_sb = singles.tile([P, 1], bf16, tag="ones_sb")
            nc.vector.memset(ones_sb[:], 1.0)
            nc.vector.tensor_copy(
                out=vaug[:, :, :, 64:65],
                in_=ones_sb[:, :, None, None].to_broadcast((P, NKC, HG, 1)),
            )

            # ---- DMA: alternate the two HWDGE queues (sync / scalar).
            # gpsimd's SWDGE is software-slow; keep it off the critical path.
            engines = [nc.sync, nc.scalar]
            rr = [0]

            def dma_rr(out_ap, in_ap):
                e = engines[rr[0] % 2]
                rr[0] += 1
                return e.dma_start(out=out_ap, in_=in_ap)

            # interleave x8 n-chunks with the wqk8 chunks in consumption
            # order so each group's inputs land just ahead of its matmuls
            morder = (0, 4, 1, 5, 2, 6, 3, 7)
            for ko2 in range(KO2):
                dma_rr(x8big[:, 0, ko2], x8[:, 0, ko2])
            for m in morder[:2]:
                dma_rr(wqk_s[:, m], wqk8[:, m])
            nc.gpsimd.dma_start(out=tri[:], in_=mask[:])
            nc.gpsimd.dma_start(out=bq_s[:], in_=bq32[:])
            nc.gpsimd.dma_start(
                out=bv_s[:], in_=bv.rearrange("p (h d) -> p h d", h=HG))
            for ko2 in range(KO2):
                dma_rr(x8big[:, 1, ko2], x8[:, 1, ko2])
            for m in morder[2:4]:
                dma_rr(wqk_s[:, m], wqk8[:, m])
            for ko2 in range(KO2):
                dma_rr(x8big[:, 2, ko2], x8[:, 2, ko2])
            for m in morder[4:]:
                dma_rr(wqk_s[:, m], wqk8[:, m])
            for ko2 in range(KO2):
                dma_rr(x8big[:, 3, ko2], x8[:, 3, ko2])
            for ko in range(KO_C):
                dma_rr(wv_s[:, ko, :], wv[:, ko, :])
            for half in range(2):
                for ko in range(KO_C):
                    dma_rr(xbbig[:, ko, half * 1024:(half + 1) * 1024],
                           xb[:, ko, half * 1024:(half + 1) * 1024])
            for ko in range(KO_G):
                dma_rr(wp_s[:, ko, :], wp[:, ko, :])

            # ---- phase A: fp8 DoubleRow QK projection ----
            def emit_qk8(m, n):
                acc = pp.tile([P, 512], f32, tag="pp")
                for ko2 in range(KO2):
                    nc.tensor.matmul(
                        acc[:],
                        lhsT=wqk_s[:, m, ko2, :, :],
                        rhs=x8big[:, n, ko2, :, :],
                        start=(ko2 == 0),
                        stop=(ko2 == KO2 - 1),
                        perf_mode=DR,
                    )
                if m < 4:
                    nc.scalar.activation(
                        QT[:, m, n * 512:(n + 1) * 512], acc[:],
                        mybir.ActivationFunctionType.Identity,
                        bias=bq_s[:, m:m + 1], scale=1.0,
                    )
                else:
                    nc.vector.tensor_copy(
                        out=KT[:, m - 4, n * 512:(n + 1) * 512], in_=acc[:])

            # ---- V projection (bf16) ----
            def emit_v(t):
                for s in emit_v_steps(t):
                    s()

            def emit_v_steps(t):
                acc = pp.tile([P, GC], f32, tag="pp")
                steps = []
                for ko in range(KO_C):
                    def mk(ko=ko, acc=acc):
                        nc.tensor.matmul(
                            acc[:],
                            lhsT=xbbig[:, ko, t * P:(t + 1) * P],
                            rhs=wv_s[:, ko, :],
                            start=(ko == 0),
                            stop=(ko == KO_C - 1),
                        )
                    steps.append(mk)

                def fin(acc=acc):
                    nc.vector.tensor_tensor(
                        vaug[:, t, :, 0:64],
                        acc[:].rearrange("p (h d) -> p h d", h=HG),
                        bv_s[:],
                        mybir.AluOpType.add,
                    )
                steps.append(fin)
                return steps

            # ---- output projection tile as single-matmul steps ----
            oq = [0]

            def emit_c_steps(t, n):
                opsum = pp.tile([P, 512], f32, tag="pp")
                steps = []
                for ko in range(KO_G):
                    def mk(ko=ko, opsum=opsum):
                        nc.tensor.matmul(
                            opsum[:],
                            lhsT=YT[:, ko, t * P:(t + 1) * P],
                            rhs=wp_s[:, ko, n * 512:(n + 1) * 512],
                            start=(ko == 0),
                            stop=(ko == KO_G - 1),
                        )
                    steps.append(mk)

                def fin(opsum=opsum):
                    osb = opool.tile([P, 512], f16, tag="osb")
                    nc.vector.tensor_copy(out=osb[:], in_=opsum[:])
                    oq[0] += 1
                    nc.sync.dma_start(
                        out=out[t * P:(t + 1) * P, n * 512:(n + 1) * 512],
                        in_=osb[:],
                    )
                steps.append(fin)
                return steps

            # ---- phase B: attention for one (q-window, head-pair) ----
            def emit_b(qbase, qlen, hp, fillers=()):
                    fillers = list(fillers)
                    nkc = (qbase + qlen) // P
                    hold = 2 if len(fillers) > 2 else 0
                    per_kc = (max(0, len(fillers) - hold) + nkc - 1) // nkc
                    fi = 0
                    ype = py.tile([P, 512], f32, tag="py")
                    ypo = py.tile([P, 512], f32, tag="py")
                    for kc in range(nkc):
                        qo = max(kc * P - qbase, 0)
                        w = qlen - qo
                        spsum = ps.tile([P, 2, 512], f32, tag="ps")
                        for odd in (0, 1):
                            po = odd * 64
                            nc.tensor.matmul(
                                spsum[:, odd, 0:w],
                                lhsT=KT[po:po + 64, hp, kc * P:(kc + 1) * P],
                                rhs=QT[po:po + 64, hp,
                                       qbase + qo:qbase + qlen],
                                start=True,
                                stop=True,
                            )
                        pt = ppool.tile([P, 2, 512], bf16, tag="pt")
                        nc.scalar.activation(
                            pt[:, :, 0:w], spsum[:, :, 0:w],
                            mybir.ActivationFunctionType.Exp,
                            scale=0.125 / 1024.0,
                        )
                        if kc * P >= qbase:
                            nc.vector.tensor_tensor(
                                pt[:, :, 0:P], pt[:, :, 0:P],
                                tri[:, None, :].to_broadcast((P, 2, P)),
                                mybir.AluOpType.mult,
                            )
                        for odd, yp in ((0, ype), (1, ypo)):
                            nc.tensor.matmul(
                                yp[0:65, qo:qlen],
                                lhsT=vaug[:, kc, 2 * hp + odd, :],
                                rhs=pt[:, odd, 0:w],
                                start=(kc == 0),
                                stop=(kc == nkc - 1),
                            )
                        for _ in range(per_kc):
                            if fi < len(fillers) - hold:
                                fillers[fi]()
                                fi += 1
                    # normalize; remaining (held) fillers keep the PE busy
                    # while the PSUM copies drain
                    for odd, yp in ((0, ype), (1, ypo)):
                        po = odd * 64
                        ycop = spool.tile([64, 512], f32, tag="ycop")
                        nc.vector.tensor_copy(
                            out=ycop[:, 0:qlen], in_=yp[0:64, 0:qlen])
                        rden = spool.tile([1, 512], f32, tag="rden")
                        nc.vector.tensor_copy(
                            out=rden[:, 0:qlen], in_=yp[64:65, 0:qlen])
                        nc.vector.reciprocal_approx_fast(
                            out=rden[:, 0:qlen], in_=rden[:, 0:qlen])
                        srep = spool.tile([64, 512], f32, tag="srep")
                        nc.gpsimd.partition_broadcast(
                            srep[:, 0:qlen], rden[:, 0:qlen])
                        if fi < len(fillers):
                            fillers[fi]()
                            fi += 1
                        yslice = YT[po:po + 64, hp, qbase:qbase + qlen]
                        if odd == 0:
                            nc.vector.tensor_tensor(
                                yslice, ycop[:, 0:qlen], srep[:, 0:qlen],
                                mybir.AluOpType.mult,
                            )
                        else:
                            # DVE lanes can't shift partitions; stage at 0..63
                            # and DMA to partitions 64..127
                            yt_tmp = spool.tile([64, 512], bf16, tag="yt_tmp")
                            nc.vector.tensor_tensor(
                                yt_tmp[:, 0:qlen], ycop[:, 0:qlen],
                                srep[:, 0:qlen],
                                mybir.AluOpType.mult,
                            )
                            nc.sync.dma_start(out=yslice, in_=yt_tmp[:, 0:qlen])
                    while fi < len(fillers):
                        fillers[fi]()
                        fi += 1

            # ---- schedule ----
            for n in range(3):
                for m in (0, 4, 1, 5, 2, 6, 3, 7):
                    emit_qk8(m, n)
            for t in range(4):
                emit_v(t)
            for hp in range(HG // 2):
                fill = list(emit_v_steps(4 + hp))
                emit_b(0, 512, hp, fillers=fill)
                m1, m2 = (0, 4, 1, 5, 2, 6, 3, 7)[hp * 2:hp * 2 + 2]
                emit_qk8(m1, 3)
                emit_qk8(m2, 3)
            for qc in (1, 2):
                for hp in range(HG // 2):
                    t = (qc - 1) * 4 + hp
                    fill = []
                    fill.extend(emit_c_steps(t, 0))
                    fill.extend(emit_c_steps(t, 1))
                    fill.extend(emit_v_steps(4 * (qc + 1) + hp))
                    emit_b(qc * 512, 512, hp, fillers=fill)
            # last q-chunk as two half-windows: t=12/13 output projections
            # overlap the second half instead of trailing the kernel
            for hp in range(HG // 2):
                fill = list(emit_c_steps(8 + hp, 0))
                emit_b(1536, 256, hp, fillers=fill)
            for hp in range(HG // 2):
                fill = list(emit_c_steps(8 + hp, 1))
                fill.extend(emit_c_steps(12 + hp // 2, hp % 2))
                emit_b(1792, 256, hp, fillers=fill)
            # tail: rows 1792..2047
            for t in (14, 15):
                for n2 in (0, 1):
                    for s in emit_c_steps(t, n2):
                        s()
    nc.finalize()
    return nc


def _get_nc():
    if "nc" not in _cache:
        _cache["nc"] = _build()
    return _cache["nc"]


def _prep_inputs(x, W_attn, b_attn, W_proj):
    import ml_dtypes
    bf = ml_dtypes.bfloat16
    f8 = ml_dtypes.float8_e4m3
    x = np.ascontiguousarray(np.asarray(x, np.float32))
    W_attn = np.asarray(W_attn, np.float32)
    b_attn = np.asarray(b_attn, np.float32)
    W_proj = np.asarray(W_proj, np.float32)
    mask = (np.arange(P)[:, None] <= np.arange(P)[None, :]).astype(bf)
    in_maps = []
    for b in range(B):
        xT = x[b].T  # [C, T]
        # [p, n, ko2, i, 512]
        x8b = np.ascontiguousarray(
            xT.reshape(KO2, 2, P, NQC, 512).transpose(2, 3, 0, 1, 4)).astype(f8)
        xbb = np.ascontiguousarray(
            xT.reshape(KO_C, P, T).transpose(1, 0, 2)).astype(bf)
        for g in range(G):
            qs, ks, vs = g * GC, C + g * GC, 2 * C + g * GC
            w2 = np.concatenate(
                [W_attn[:, qs:qs + GC], W_attn[:, ks:ks + GC]], 1) * 32.0
            # [C, 1024] -> [128 p, 8 m, 4 ko2, 2 i, 128 col]
            wq8 = np.ascontiguousarray(
                w2.reshape(KO2, 2, P, 8, P).transpose(2, 3, 0, 1, 4)).astype(f8)
            in_maps.append({
                "x8": x8b,
                "xb": xbb,
                "wqk8": wq8,
                "bq32": np.ascontiguousarray(
                    32.0 * b_attn[qs:qs + GC].reshape(4, P).T),
                "wv": np.ascontiguousarray(
                    W_attn[:, vs:vs + GC].reshape(KO_C, P, GC)
                    .transpose(1, 0, 2)).astype(bf),
                "bv": np.ascontiguousarray(
                    np.broadcast_to(b_attn[vs:vs + GC], (P, GC))),
                "wp": np.ascontiguousarray(
                    W_proj[g * GC:(g + 1) * GC, :].reshape(KO_G, P, C)
                    .transpose(1, 0, 2)).astype(bf),
                "mask": mask,
            })
    return in_maps


def _run(inputs, trace=False):
    from concourse.bass_utils import run_bass_kernel_spmd

    nc = _get_nc()
    in_maps = _prep_inputs(
        inputs["x"], inputs["W_attn"], inputs["b_attn"], inputs["W_proj"]
    )
    res = run_bass_kernel_spmd(nc, in_maps, list(range(B * G)), trace=trace)
    b_proj = np.asarray(inputs["b_proj"], np.float32)
    outs = [
        res.results[2 * b]["out"].astype(np.float32)
        + res.results[2 * b + 1]["out"].astype(np.float32) + b_proj
        for b in range(B)
    ]
    return np.stack(outs).astype(np.float32), res


def kernel(**inputs):
    return _run(inputs, trace=False)[0]


if __name__ == "__main__":
    rng = np.random.default_rng(0)
    ins = {
        "x": rng.standard_normal((B, T, C), np.float32),
        "W_attn": rng.uniform(-0.03, 0.03, (C, 3 * C)).astype(np.float32),
        "b_attn": rng.uniform(-0.03, 0.03, (3 * C,)).astype(np.float32),
        "W_proj": rng.uniform(-0.03, 0.03, (C, C)).astype(np.float32),
        "b_proj": rng.uniform(-0.03, 0.03, (C,)).astype(np.float32),
    }
    out = kernel(**ins)
    print("ran, out shape", out.shape)


# revision 10
# speedup vs baseline: 1.0611x; 1.0611x over previous
"""Causal self-attention (B=4, T=2048, C=1024, H=16, HD=64) on 8 trn2 cores.

Sharding: core = (batch b, head-group g), 8 heads each. Host sums the two
partial output projections per batch and adds b_proj.

v4 kernel (per core):
  A: QK^T projection in fp8e4 + DoubleRow (contraction 256/instr, 2x):
     psum = 32*(x Wqk) (weights pre-scaled x32 on host to dodge e4m3
     subnormals; K bias dropped -- softmax-invariant; Q bias x32 via ACT).
     QT/KT stored bf16 at 32x scale -> exp scale absorbs /1024.
     V projection bf16 -> vaug bf16 [k,(h,d+ones)].
  B: per (q-window, head-pair): S^T[k,q] bf16 matmul pair on PE row-groups
     h0/h64 (runs concurrent); one wide exp (scale 0.125/1024) -> pt bf16;
     diagonal blocks width-reduced + fixed lower-triangle mask; AV bf16
     accumulates per head in PSUM, ones column = softmax denominator row.
     Normalize: DVE copies (tensor_copy shifts partitions; other DVE ops
     can't), recip, gpsimd partition-broadcast, DVE mult -> YT bf16.
  C: out = Y @ Wp (bf16), PSUM -> fp16 SBUF -> HBM; host sums in fp32.
Schedule: input DMAs round-robin over the 4 DGE queues (sync/scalar/
vector/gpsimd) -- one queue is only ~45GB/s; n-major QK groups start the
PE ~3us in; qc0 attention + V proj interleave with late QK groups;
out-projection/V matmuls injected at kc granularity inside attention
loops (2 held back past the last AV to cover the PSUM copy-out); the
last q-chunk runs as two half-windows so its output projection
overlaps attention instead of trailing.
"""

import numpy as np

B, T, C, H, HD = 4, 2048, 1024, 16, 64
G = 2              # head groups (tensor parallel)
HG = H // G        # 8 heads per group
GC = HG * HD       # 512 group channels
P = 128
NQC = T // 512     # 4 q-chunks of 512
NKC = T // P       # 16 k-chunks of 128
KO_C = C // P      # 8 contraction chunks for C=1024
KO2 = C // 256     # 4 DoubleRow contraction chunks
KO_G = GC // P     # 4 contraction chunks for GC=512

_cache = {}


def _build():
    import concourse.bass as bass
    import concourse.tile as tile
    from concourse import bacc, mybir

    f32 = mybir.dt.float32
    f16 = mybir.dt.float16
    bf16 = mybir.dt.bfloat16
    f8 = mybir.dt.float8e4
    DR = mybir.MatmulPerfMode.DoubleRow

    nc = bacc.Bacc(name="csa")
    x8 = nc.declare_dram_parameter("x8", [P, NQC, KO2, 2, 512], f8, isOutput=False)
    xb = nc.declare_dram_parameter("xb", [P, KO_C, T], bf16, isOutput=False)
    wqk8 = nc.declare_dram_parameter("wqk8", [P, 8, KO2, 2, P], f8, isOutput=False)
    bq32 = nc.declare_dram_parameter("bq32", [P, 4], f32, isOutput=False)
    wv = nc.declare_dram_parameter("wv", [P, KO_C, GC], bf16, isOutput=False)
    bv = nc.declare_dram_parameter("bv", [P, GC], f32, isOutput=False)
    wp = nc.declare_dram_parameter("wp", [P, KO_G, C], bf16, isOutput=False)
    mask = nc.declare_dram_parameter("mask", [P, P], bf16, isOutput=False)
    out = nc.declare_dram_parameter("out", [T, C], f16, isOutput=True)

    from contextlib import ExitStack

    with tile.TileContext(nc) as tc, ExitStack() as ctx:
            singles = ctx.enter_context(tc.tile_pool(name="singles", bufs=1))
            ppool = ctx.enter_context(tc.tile_pool(name="ppool", bufs=3))
            spool = ctx.enter_context(tc.tile_pool(name="spool", bufs=4))
            opool = ctx.enter_context(tc.tile_pool(name="opool", bufs=3))
            pp = ctx.enter_context(tc.tile_pool(name="pp", bufs=2, space="PSUM"))
            ps = ctx.enter_context(tc.tile_pool(name="ps", bufs=2, space="PSUM"))
            py = ctx.enter_context(tc.tile_pool(name="py", bufs=2, space="PSUM"))
            # ---- resident tensors ----
            x8big = singles.tile([P, NQC, KO2, 2, 512], f8, tag="x8big")
            xbbig = singles.tile([P, KO_C, T], bf16, tag="xbbig")
            wqk_s = singles.tile([P, 8, KO2, 2, P], f8, tag="wqk_s")
            QT = singles.tile([P, HG // 2, T], bf16, tag="QT")
            KT = singles.tile([P, HG // 2, T], bf16, tag="KT")
            vaug = singles.tile([P, NKC, HG, 65], bf16, tag="vaug")
            YT = singles.tile([P, KO_G, T], bf16, tag="YT")
            tri = singles.tile([P, P], bf16, tag="tri")
            bq_s = singles.tile([P, 4], f32, tag="bq")
            bv_s = singles.tile([P, HG, HD], f32, tag="bv")
            wv_s = singles.tile([P, KO_C, GC], bf16, tag="wv")
            wp_s = singles.tile([P, KO_G, C], bf16, tag="wp")

            ones_sb = singles.tile([P, 1], bf16, tag="ones_sb")
            nc.vector.memset(ones_sb[:], 1.0)
            nc.vector.tensor_copy(
                out=vaug[:, :, :, 64:65],
                in_=ones_sb[:, :, None, None].to_broadcast((P, NKC, HG, 1)),
            )

            # ---- DMA: all input DMAs on the sync (SP) HWDGE queue, in
            # consumption order. DMAs issued from scalar/vector would sit
            # IN FRONT of that engine's compute in its sequencer and starve
            # the PSUM copy-outs the PE waits on; gpsimd's SWDGE is slow,
            # so it only gets small constants consumed late.
            def dma_rr(out_ap, in_ap):
                return nc.sync.dma_start(out=out_ap, in_=in_ap)

            morder = (0, 4, 1, 5, 2, 6, 3, 7)
            for ko2 in range(KO2):
                dma_rr(x8big[:, 0, ko2], x8[:, 0, ko2])
            nc.sync.dma_start(out=bq_s[:], in_=bq32[:])
            for m in morder:
                dma_rr(wqk_s[:, m], wqk8[:, m])
            nc.gpsimd.dma_start(out=tri[:], in_=mask[:])
            nc.gpsimd.dma_start(
                out=bv_s[:], in_=bv.rearrange("p (h d) -> p h d", h=HG))
            for n in range(1, NQC):
                for ko2 in range(KO2):
                    dma_rr(x8big[:, n, ko2], x8[:, n, ko2])
            for ko in range(KO_C):
                dma_rr(wv_s[:, ko, :], wv[:, ko, :])
            for half in range(2):
                for ko in range(KO_C):
                    dma_rr(xbbig[:, ko, half * 1024:(half + 1) * 1024],
                           xb[:, ko, half * 1024:(half + 1) * 1024])
            for ko in range(KO_G):
                dma_rr(wp_s[:, ko, :], wp[:, ko, :])

            # ---- phase A: fp8 DoubleRow QK projection ----
            def emit_qk8(m, n):
                acc = pp.tile([P, 512], f32, tag="pp")
                for ko2 in range(KO2):
                    nc.tensor.matmul(
                        acc[:],
                        lhsT=wqk_s[:, m, ko2, :, :],
                        rhs=x8big[:, n, ko2, :, :],
                        start=(ko2 == 0),
                        stop=(ko2 == KO2 - 1),
                        perf_mode=DR,
                    )
                if m < 4:
                    nc.scalar.activation(
                        QT[:, m, n * 512:(n + 1) * 512], acc[:],
                        mybir.ActivationFunctionType.Identity,
                        bias=bq_s[:, m:m + 1], scale=1.0,
                    )
                else:
                    nc.vector.tensor_copy(
                        out=KT[:, m - 4, n * 512:(n + 1) * 512], in_=acc[:])

            # ---- V projection (bf16) ----
            def emit_v(t):
                for s in emit_v_steps(t):
                    s()

            def emit_v_steps(t):
                acc = pp.tile([P, GC], f32, tag="pp")
                steps = []
                for ko in range(KO_C):
                    def mk(ko=ko, acc=acc):
                        nc.tensor.matmul(
                            acc[:],
                            lhsT=xbbig[:, ko, t * P:(t + 1) * P],
                            rhs=wv_s[:, ko, :],
                            start=(ko == 0),
                            stop=(ko == KO_C - 1),
                        )
                    steps.append(mk)

                def fin(acc=acc):
                    nc.vector.tensor_tensor(
                        vaug[:, t, :, 0:64],
                        acc[:].rearrange("p (h d) -> p h d", h=HG),
                        bv_s[:],
                        mybir.AluOpType.add,
                    )
                steps.append(fin)
                return steps

            # ---- output projection tile as single-matmul steps ----
            oq = [0]

            def emit_c_steps(t, n):
                opsum = pp.tile([P, 512], f32, tag="pp")
                steps = []
                for ko in range(KO_G):
                    def mk(ko=ko, opsum=opsum):
                        nc.tensor.matmul(
                            opsum[:],
                            lhsT=YT[:, ko, t * P:(t + 1) * P],
                            rhs=wp_s[:, ko, n * 512:(n + 1) * 512],
                            start=(ko == 0),
                            stop=(ko == KO_G - 1),
                        )
                    steps.append(mk)

                def fin(opsum=opsum):
                    osb = opool.tile([P, 512], f16, tag="osb")
                    nc.vector.tensor_copy(out=osb[:], in_=opsum[:])
                    oq[0] += 1
                    nc.sync.dma_start(
                        out=out[t * P:(t + 1) * P, n * 512:(n + 1) * 512],
                        in_=osb[:],
                    )
                steps.append(fin)
                return steps

            # ---- phase B: attention for one (q-window, head-pair) ----
            def emit_b(qbase, qlen, hp, fillers=()):
                    fillers = list(fillers)
                    nkc = (qbase + qlen) // P
                    hold = 2 if len(fillers) > 2 else 0
                    per_kc = (max(0, len(fillers) - hold) + nkc - 1) // nkc
                    fi = 0
                    ype = py.tile([P, 512], f32, tag="py")
                    ypo = py.tile([P, 512], f32, tag="py")
                    for kc in range(nkc):
                        qo = max(kc * P - qbase, 0)
                        w = qlen - qo
                        spsum = ps.tile([P, 2, 512], f32, tag="ps")
                        for odd in (0, 1):
                            po = odd * 64
                            nc.tensor.matmul(
                                spsum[:, odd, 0:w],
                                lhsT=KT[po:po + 64, hp, kc * P:(kc + 1) * P],
                                rhs=QT[po:po + 64, hp,
                                       qbase + qo:qbase + qlen],
                                start=True,
                                stop=True,
                            )
                        pt = ppool.tile([P, 2, 512], bf16, tag="pt")
                        nc.scalar.activation(
                            pt[:, :, 0:w], spsum[:, :, 0:w],
                            mybir.ActivationFunctionType.Exp,
                            scale=0.125 / 1024.0,
                        )
                        if kc * P >= qbase:
                            nc.vector.tensor_tensor(
                                pt[:, :, 0:P], pt[:, :, 0:P],
                                tri[:, None, :].to_broadcast((P, 2, P)),
                                mybir.AluOpType.mult,
                            )
                        for odd, yp in ((0, ype), (1, ypo)):
                            nc.tensor.matmul(
                                yp[0:65, qo:qlen],
                                lhsT=vaug[:, kc, 2 * hp + odd, :],
                                rhs=pt[:, odd, 0:w],
                                start=(kc == 0),
                                stop=(kc == nkc - 1),
                            )
                        for _ in range(per_kc):
                            if fi < len(fillers) - hold:
                                fillers[fi]()
                                fi += 1
                    # normalize; remaining (held) fillers keep the PE busy
                    # while the PSUM copies drain
                    for odd, yp in ((0, ype), (1, ypo)):
                        po = odd * 64
                        ycop = spool.tile([64, 512], f32, tag="ycop")
                        nc.vector.tensor_copy(
                            out=ycop[:, 0:qlen], in_=yp[0:64, 0:qlen])
                        rden = spool.tile([1, 512], f32, tag="rden")
                        nc.vector.tensor_copy(
                            out=rden[:, 0:qlen], in_=yp[64:65, 0:qlen])
                        nc.vector.reciprocal_approx_fast(
                            out=rden[:, 0:qlen], in_=rden[:, 0:qlen])
                        srep = spool.tile([64, 512], f32, tag="srep")
                        nc.gpsimd.partition_broadcast(
                            srep[:, 0:qlen], rden[:, 0:qlen])
                        if fi < len(fillers):
                            fillers[fi]()
                            fi += 1
                        yslice = YT[po:po + 64, hp, qbase:qbase + qlen]
                        if odd == 0:
                            nc.vector.tensor_tensor(
                                yslice, ycop[:, 0:qlen], srep[:, 0:qlen],
                                mybir.AluOpType.mult,
                            )
                        else:
                            # DVE lanes can't shift partitions; stage at 0..63
                            # and DMA to partitions 64..127
                            yt_tmp = spool.tile([64, 512], bf16, tag="yt_tmp")
                            nc.vector.tensor_tensor(
                                yt_tmp[:, 0:qlen], ycop[:, 0:qlen],
                                srep[:, 0:qlen],
                                mybir.AluOpType.mult,
                            )
                            nc.sync.dma_start(out=yslice, in_=yt_tmp[:, 0:qlen])
                    while fi < len(fillers):
                        fillers[fi]()
                        fi += 1

            # ---- schedule ----
            for n in range(3):
                for m in (0, 4, 1, 5, 2, 6, 3, 7):
                    emit_qk8(m, n)
            for t in range(4):
                emit_v(t)
            for hp in range(HG // 2):
                fill = list(emit_v_steps(4 + hp))
                emit_b(0, 512, hp, fillers=fill)
                m1, m2 = (0, 4, 1, 5, 2, 6, 3, 7)[hp * 2:hp * 2 + 2]
                emit_qk8(m1, 3)
                emit_qk8(m2, 3)
            for qc in (1, 2):
                for hp in range(HG // 2):
                    t = (qc - 1) * 4 + hp
                    fill = []
                    fill.extend(emit_c_steps(t, 0))
                    fill.extend(emit_c_steps(t, 1))
                    fill.extend(emit_v_steps(4 * (qc + 1) + hp))
                    emit_b(qc * 512, 512, hp, fillers=fill)
            # last q-chunk as two half-windows: t=12/13 output projections
            # overlap the second half instead of trailing the kernel
            for hp in range(HG // 2):
                fill = list(emit_c_steps(8 + hp, 0))
                emit_b(1536, 256, hp, fillers=fill)
            for hp in range(HG // 2):
                fill = list(emit_c_steps(8 + hp, 1))
                fill.extend(emit_c_steps(12 + hp // 2, hp % 2))
                emit_b(1792, 256, hp, fillers=fill)
            # tail: rows 1792..2047
            for t in (14, 15):
                for n2 in (0, 1):
                    for s in emit_c_steps(t, n2):
                        s()
    nc.finalize()
    return nc


def _get_nc():
    if "nc" not in _cache:
        _cache["nc"] = _build()
    return _cache["nc"]


def _prep_inputs(x, W_attn, b_attn, W_proj):
    import ml_dtypes
    bf = ml_dtypes.bfloat16
    f8 = ml_dtypes.float8_e4m3
    x = np.ascontiguousarray(np.asarray(x, np.float32))
    W_attn = np.asarray(W_attn, np.float32)
    b_attn = np.asarray(b_attn, np.float32)
    W_proj = np.asarray(W_proj, np.float32)
    mask = (np.arange(P)[:, None] <= np.arange(P)[None, :]).astype(bf)
    in_maps = []
    for b in range(B):
        xT = x[b].T  # [C, T]
        # [p, n, ko2, i, 512]
        x8b = np.ascontiguousarray(
            xT.reshape(KO2, 2, P, NQC, 512).transpose(2, 3, 0, 1, 4)).astype(f8)
        xbb = np.ascontiguousarray(
            xT.reshape(KO_C, P, T).transpose(1, 0, 2)).astype(bf)
        for g in range(G):
            qs, ks, vs = g * GC, C + g * GC, 2 * C + g * GC
            w2 = np.concatenate(
                [W_attn[:, qs:qs + GC], W_attn[:, ks:ks + GC]], 1) * 32.0
            # [C, 1024] -> [128 p, 8 m, 4 ko2, 2 i, 128 col]
            wq8 = np.ascontiguousarray(
                w2.reshape(KO2, 2, P, 8, P).transpose(2, 3, 0, 1, 4)).astype(f8)
            in_maps.append({
                "x8": x8b,
                "xb": xbb,
                "wqk8": wq8,
                "bq32": np.ascontiguousarray(
                    32.0 * b_attn[qs:qs + GC].reshape(4, P).T),
                "wv": np.ascontiguousarray(
                    W_attn[:, vs:vs + GC].reshape(KO_C, P, GC)
                    .transpose(1, 0, 2)).astype(bf),
                "bv": np.ascontiguousarray(
                    np.broadcast_to(b_attn[vs:vs + GC], (P, GC))),
                "wp": np.ascontiguousarray(
                    W_proj[g * GC:(g + 1) * GC, :].reshape(KO_G, P, C)
                    .transpose(1, 0, 2)).astype(bf),
                "mask": mask,
            })
    return in_maps


def _run(inputs, trace=False):
    from concourse.bass_utils import run_bass_kernel_spmd

    nc = _get_nc()
    in_maps = _prep_inputs(
        inputs["x"], inputs["W_attn"], inputs["b_attn"], inputs["W_proj"]
    )
    res = run_bass_kernel_spmd(nc, in_maps, list(range(B * G)), trace=trace)
    b_proj = np.asarray(inputs["b_proj"], np.float32)
    outs = [
        res.results[2 * b]["out"].astype(np.float32)
        + res.results[2 * b + 1]["out"].astype(np.float32) + b_proj
        for b in range(B)
    ]
    return np.stack(outs).astype(np.float32), res


def kernel(**inputs):
    return _run(inputs, trace=False)[0]


if __name__ == "__main__":
    rng = np.random.default_rng(0)
    ins = {
        "x": rng.standard_normal((B, T, C), np.float32),
        "W_attn": rng.uniform(-0.03, 0.03, (C, 3 * C)).astype(np.float32),
        "b_attn": rng.uniform(-0.03, 0.03, (3 * C,)).astype(np.float32),
        "W_proj": rng.uniform(-0.03, 0.03, (C, C)).astype(np.float32),
        "b_proj": rng.uniform(-0.03, 0.03, (C,)).astype(np.float32),
    }
    out = kernel(**ins)
    print("ran, out shape", out.shape)
